# revision 1
# baseline (speedup 1.0000x reference)
"""Trainium2 Bass kernel for causal attention + proj + causal_features.

Problem shapes: x [2, 2048, 1024], H=16 heads, HD=64.
Strategy (8 NeuronCores):
  - Head-parallel attention: core i computes QKV + attention for heads {2i, 2i+1}
    over both batches, everything in transposed [channel, token] layout so the
    contraction dim sits on SBUF partitions.
  - Scores computed as S^T[k, q] = K @ Q^T (contraction d=64); exp on ScalarE
    (scale=1/8 folded in); causal mask applied only on diagonal-band tiles via
    a precomputed 0/1 multiplier; upper-triangle tiles skipped entirely.
  - attn @ V via lhsT = [V | 1] (ones column) so the softmax denominators fall
    out of the same matmul as row 64 of the PSUM accumulator.
  - AllToAll reshards attention output from head-parallel to token-parallel;
    each core then computes proj + bias and causal_features for its 512-token
    slice. Outputs returned transposed [1024, 512] and re-assembled on host.
  - All matmuls run in float32r (full PE rate at free dim >= 256, ~1e-4 rel err).
"""

import numpy as np
import ml_dtypes  # noqa: F401  (registers bfloat16 with numpy)
import concourse.bass as bass
import concourse.mybir as mybir
import concourse.tile as tile
from concourse import bacc
from concourse.bass_utils import run_bass_kernel_spmd

B, N, C, H, HD = 2, 2048, 1024, 16, 64
NCORES = 8
TOK = B * N            # 4096 global tokens
TPC = TOK // NCORES    # 512 tokens per core (output slice)
QC = 512               # q chunk width
KTILE = 128            # k tile height
NKT = N // KTILE       # 16 k tiles per batch
NQC = N // QC          # 4 q chunks per batch
f32 = mybir.dt.float32
f32r = mybir.dt.float32r
bf16 = mybir.dt.bfloat16
AF = mybir.ActivationFunctionType
ALU = mybir.AluOpType

_CACHE = {}


def _build_program(fake_a2a=False, reps=1, phase=99, av_delay=False, pe_norm=False):
    nc = bacc.Bacc("TRN2", target_bir_lowering=False, debug=False, num_devices=NCORES)

    xT_d = nc.dram_tensor("xT", [C, TOK], f32r, kind="ExternalInput")
    wqkv_d = nc.dram_tensor("wqkv", [128, 3 * 8 * 128], f32r, kind="ExternalInput")
    wp_d = nc.dram_tensor("wp", [8, 128, 8 * 128], f32r, kind="ExternalInput")
    wc_d = nc.dram_tensor("wc", [8, 128, 8 * 128], f32r, kind="ExternalInput")
    bias_d = nc.dram_tensor("bias", [128, 16], f32, kind="ExternalInput")
    maskt_d = nc.dram_tensor("maskt", [128, 4 * QC], f32, kind="ExternalInput")
    ident_d = nc.dram_tensor("ident", [128, 64], f32r, kind="ExternalInput")
    outT_d = nc.dram_tensor("outT", [C, TPC], f32r, kind="ExternalOutput")
    czT_d = nc.dram_tensor("czT", [C, TPC], f32r, kind="ExternalOutput")

    with tile.TileContext(nc) as tc:
        with tc.tile_pool(name="sb", bufs=1) as sb, \
             tc.tile_pool(name="ps", bufs=1, space="PSUM") as ps, \
             tc.tile_pool(name="dr", bufs=1, space="DRAM") as dr:

            # ---- constants (wqkv on the HWDGE queue ahead of the xT stream;
            #      the rest on SWDGE so they don't delay it) ----
            wqkv_sb = sb.tile([128, 3 * 8 * 128], f32r)     # [c_in 128][(m,kt,c)]
            for mq in range(3):
                nc.gpsimd.dma_start(wqkv_sb[:, mq * 1024:(mq + 1) * 1024],
                                    wqkv_d[:, mq * 1024:(mq + 1) * 1024])
            maskt_sb = sb.tile([128, 4 * QC], f32)
            nc.gpsimd.dma_start(maskt_sb[:], maskt_d[:])
            ident_sb = sb.tile([128, 64], f32r)
            nc.gpsimd.dma_start(ident_sb[:], ident_d[:])
            bias_sb = sb.tile([128, 16], f32)               # cols 0:8 = bp tiles, 8:16 = bc
            nc.gpsimd.dma_start(bias_sb[:], bias_d[:])
            ones64 = sb.tile([1, 64], f32r)
            nc.vector.tensor_scalar(ones64[:], maskt_sb[0:1, 0:64], 0.0, 1.0, ALU.mult, ALU.add)

            for _rep in range(reps):
                qT_sb = sb.tile([128, TOK], f32r, tag="qT", bufs=1, name="qT_sb")
                kT_sb = sb.tile([128, TOK], f32r, tag="kT", bufs=1, name="kT_sb")
                # rotating 16KB scratch: vT -> outT -> otf -> pj -> cz
                vT_sb = sb.tile([128, TOK], f32r, tag="scratch16", bufs=2, name="vT_sb")
                v_aug = sb.tile([128, B * 2 * NKT * 65], f32r, tag="v_aug", bufs=1, name="v_aug")
                outT_sb = sb.tile([128, TOK], f32r, tag="scratch16", bufs=2, name="outT_sb")
                # ones columns of v_aug, written once up front
                nc.vector.tensor_scalar(v_aug[:, 64::65], maskt_sb[:, 0:B * 2 * NKT],
                                        0.0, 1.0, ALU.mult, ALU.add)

                wpms, wcms = {}, {}
                a2a_in = dr.tile([NCORES, 128, TPC], f32r, name="a2a_in")
                a2a_out = dr.tile([NCORES, 128, TPC], f32r, name="a2a_out")
                # ---- fused pipeline over token chunks: QKV(n) -> vT(n) -> attention(b,j=n) ----
                for n in range(TOK // QC):
                    b, j = n // NQC, n % NQC
                    # QKV for chunk n, kt-outer so the xT stream overlaps compute
                    accs = [ps.tile([128, QC], f32, tag="mm", bufs=3, name=f"qkv_{n}_{m}")
                            for m in range(3)]
                    for kt in range(8):
                        xt = sb.tile([128, QC], f32r, tag="xt", bufs=10, name=f"xt_{n}_{kt}")
                        dma_eng = nc.sync if kt % 2 == 0 else nc.scalar
                        dma_eng.dma_start(xt[:], xT_d[kt * 128:(kt + 1) * 128, n * QC:(n + 1) * QC])
                        for m in range(3):
                            nc.tensor.matmul(accs[m][:],
                                             wqkv_sb[:, (m * 8 + kt) * 128:(m * 8 + kt + 1) * 128],
                                             xt[:], start=(kt == 0), stop=(kt == 7))
                    nc.scalar.activation(qT_sb[:, n * QC:(n + 1) * QC], accs[0][:], AF.Copy)
                    nc.vector.tensor_copy(kT_sb[:, n * QC:(n + 1) * QC], accs[1][:])
                    nc.vector.tensor_copy(vT_sb[:, n * QC:(n + 1) * QC], accs[2][:])
                    # V transposes for this chunk
                    for h in range(2):
                        for kt in range(4 * j, 4 * j + 4):
                            tp = ps.tile([128, 64], f32r, tag="mm", bufs=3, name=f"vt_{n}_{h}_{kt}")
                            nc.tensor.transpose(
                                tp[:], vT_sb[h * 64:(h + 1) * 64,
                                             b * N + kt * 128:b * N + (kt + 1) * 128],
                                ident_sb[h * 64:(h + 1) * 64, :])
                            slot = ((b * 2 + h) * NKT + kt) * 65
                            nc.vector.tensor_copy(v_aug[:, slot:slot + 64], tp[:])
                    if phase < 1:
                        continue
                    # attention for (b, j): heads sequential; AV trails scores by one
                    # group so PE never in-order-blocks on the exp it needs
                    qc0 = b * N + j * QC
                    last_kt = 4 * j + 3
                    for h in range(2):
                        av = ps.tile([65, QC], f32, tag="av", bufs=1, name=f"av_{n}_{h}")
                        ngroups = 2 * j + 2
                        exs = {}
                        for gi in range(ngroups + 1):
                            if gi < ngroups:
                                kt0 = 2 * gi
                                sc2 = ps.tile([128, 2 * QC], f32, tag="sc", bufs=2,
                                              name=f"sc_{n}_{gi}_{h}")
                                for t in range(2):
                                    kk = b * N + (kt0 + t) * 128
                                    nc.tensor.matmul(sc2[:, t * QC:(t + 1) * QC],
                                                     kT_sb[h * 64:(h + 1) * 64, kk:kk + 128],
                                                     qT_sb[h * 64:(h + 1) * 64, qc0:qc0 + QC],
                                                     start=True, stop=True)
                                ex2 = sb.tile([128, 2 * QC], f32r, tag="ex", bufs=6,
                                              name=f"ex_{n}_{gi}_{h}")
                                nc.scalar.activation(ex2[:], sc2[:], AF.Exp, scale=HD ** -0.5)
                                if kt0 >= 4 * j:
                                    off = (kt0 - 4 * j) * QC
                                    nc.vector.tensor_tensor(ex2[:], ex2[:],
                                                            maskt_sb[:, off:off + 2 * QC], ALU.mult)
                                exs[gi] = ex2
                            gav = gi - 1 if av_delay else gi
                            if 0 <= gav < ngroups and (av_delay or gi < ngroups):
                                exa = exs.pop(gav)
                                for t in range(2):
                                    kt = 2 * gav + t
                                    slot = ((b * 2 + h) * NKT + kt) * 65
                                    nc.tensor.matmul(av[:], v_aug[:, slot:slot + 65],
                                                     exa[:, t * QC:(t + 1) * QC],
                                                     start=(kt == 0), stop=(kt == last_kt))
                        rc = sb.tile([1, QC], f32r, tag="rc", bufs=2, name=f"rc_{n}_{h}")
                        with nc.allow_low_precision(reason="softmax denom reciprocal"):
                            nc.vector.reciprocal(rc[:], av[64:65, :])
                        if pe_norm:
                            sclp = ps.tile([64, QC], f32, tag="mm", bufs=3, name=f"sclp_{n}_{h}")
                            nc.tensor.matmul(sclp[:], ones64[:], rc[:], start=True, stop=True)
                            rawv = sb.tile([64, QC], f32, tag="rawv", bufs=2, name=f"rawv_{n}_{h}")
                            nc.vector.tensor_copy(rawv[:], av[0:64, :])
                            nc.vector.tensor_tensor(outT_sb[h * 64:(h + 1) * 64, qc0:qc0 + QC],
                                                    rawv[:], sclp[:], ALU.mult)
                        else:
                            rb = sb.tile([64, QC], f32r, tag="rb", bufs=2, name=f"rb_{n}_{h}")
                            nc.gpsimd.partition_broadcast(rb[:], rc[:])
                            nc.vector.tensor_tensor(outT_sb[h * 64:(h + 1) * 64, qc0:qc0 + QC],
                                                    av[0:64, :], rb[:], ALU.mult)
                    if phase >= 2:
                        # stage this chunk's A2A slice immediately (chunk n == dest core n)
                        nc.gpsimd.dma_start(a2a_in[n], outT_sb[:, n * TPC:(n + 1) * TPC])
                        if n == 5:
                            # prefetch proj/causal weights under the attention tail
                            for m in range(8):
                                wpm = sb.tile([128, 1024], f32r, tag="wp", bufs=8, name=f"wpm_{m}")
                                nc.gpsimd.dma_start(wpm[:], wp_d[m])
                                wpms[m] = wpm
                            for m in range(4):
                                wcm = sb.tile([128, 1024], f32r, tag="wc", bufs=4, name=f"wcm_{m}")
                                nc.gpsimd.dma_start(wcm[:], wc_d[m])
                                wcms[m] = wcm

                if phase < 2:
                    continue
                # ---- AllToAll: head-parallel -> token-parallel (slices staged per chunk) ----
                if fake_a2a:
                    nc.sync.dma_start(a2a_out[:], a2a_in[:])
                else:
                    nc.gpsimd.collective_compute("AllToAll", ALU.bypass,
                                                 replica_groups=[list(range(NCORES))],
                                                 ins=[a2a_in.opt()], outs=[a2a_out.opt()])
                otf = sb.tile([128, NCORES * TPC], f32r, tag="scratch16", bufs=2, name="otf")
                for d in range(NCORES):
                    nc.sync.dma_start(otf[:, d * TPC:(d + 1) * TPC], a2a_out[d])

                if phase < 3:
                    continue
                # ---- proj + bias (projT [c_out, tok]) ----
                pj_sb = sb.tile([128, C // 128 * TPC], f32r, tag="scratch16", bufs=2, name="pj_sb")
                for m in range(8):
                    wpm = wpms[m]
                    acc = ps.tile([128, TPC], f32, tag="sc", bufs=2, name=f"pj_ps_{m}")
                    for kt in range(8):
                        nc.tensor.matmul(acc[:], wpm[:, kt * 128:(kt + 1) * 128],
                                         otf[:, kt * TPC:(kt + 1) * TPC],
                                         start=(kt == 0), stop=(kt == 7))
                    nc.vector.tensor_scalar_add(pj_sb[:, m * TPC:(m + 1) * TPC], acc[:],
                                                bias_sb[:, m:m + 1])
                    nc.sync.dma_start(outT_d[m * 128:(m + 1) * 128, :], pj_sb[:, m * TPC:(m + 1) * TPC])

                # ---- causal_features + bias ----
                cz_sb = sb.tile([128, C // 128 * TPC], f32r, tag="scratch16", bufs=2, name="cz_sb")
                for m in range(8):
                    if m in wcms:
                        wcm = wcms[m]
                    else:
                        wcm = sb.tile([128, 1024], f32r, tag="wc", bufs=4, name=f"wcm_{m}")
                        nc.gpsimd.dma_start(wcm[:], wc_d[m])
                    acc = ps.tile([128, TPC], f32, tag="sc", bufs=2, name=f"cz_ps_{m}")
                    for kt in range(8):
                        nc.tensor.matmul(acc[:], wcm[:, kt * 128:(kt + 1) * 128],
                                         pj_sb[:, kt * TPC:(kt + 1) * TPC],
                                         start=(kt == 0), stop=(kt == 7))
                    nc.vector.tensor_scalar_add(cz_sb[:, m * TPC:(m + 1) * TPC], acc[:],
                                                bias_sb[:, 8 + m:9 + m])
                    nc.sync.dma_start(czT_d[m * 128:(m + 1) * 128, :], cz_sb[:, m * TPC:(m + 1) * TPC])

    nc.finalize()
    return nc


def _pack_w(w):
    # [kt*128+p, m*128+c] -> [m, p, kt*128+c] contiguous per m-slice
    w = np.asarray(w, dtype=np.float32).reshape(8, 128, 8, 128)
    return np.ascontiguousarray(w.transpose(2, 1, 0, 3).reshape(8, 128, 1024))


def _host_inputs(x, mask, W_qkv, W_proj, b_proj, W_causal, b_causal):
    x = np.asarray(x, dtype=np.float32)
    xT = np.ascontiguousarray(x.reshape(TOK, C).T)
    m2 = np.asarray(mask).reshape(N, N)
    # diagonal-band mask multiplier tiles in S^T [k, q] layout, offsets d0 = k0-q0
    q0 = N - QC
    tiles = []
    for d0 in (0, 128, 256, 384):
        k0 = q0 + d0
        tiles.append(np.ascontiguousarray(m2[q0:q0 + QC, k0:k0 + 128].T.astype(np.float32)))
    maskt = np.concatenate(tiles, axis=1)
    ident = np.ascontiguousarray(np.concatenate([np.eye(64, dtype=np.float32)] * 2, axis=0))
    W_qkv = np.asarray(W_qkv, dtype=np.float32)
    shared = {
        "xT": xT,
        "wp": _pack_w(W_proj),
        "wc": _pack_w(W_causal),
        "bias": np.ascontiguousarray(np.stack(
            [np.asarray(b_proj, np.float32).reshape(8, 128),
             np.asarray(b_causal, np.float32).reshape(8, 128)]).transpose(2, 0, 1).reshape(128, 16)),
        "maskt": maskt,
        "ident": ident,
    }
    in_maps = []
    for i in range(NCORES):
        im = dict(shared)
        sl = np.stack([W_qkv[:, m * C + i * 128:m * C + (i + 1) * 128] for m in range(3)])
        # [m, kt*128+p, c] -> [p, m, kt, c]
        sl = sl.reshape(3, 8, 128, 128).transpose(2, 0, 1, 3).reshape(128, 3 * 8 * 128)
        im["wqkv"] = np.ascontiguousarray(sl)
        in_maps.append(im)
    return in_maps


def kernel(x, mask, W_qkv, W_proj, b_proj, W_causal, b_causal):
    if "nc" not in _CACHE:
        _CACHE["nc"] = _build_program()
    nc = _CACHE["nc"]
    in_maps = _host_inputs(x, mask, W_qkv, W_proj, b_proj, W_causal, b_causal)
    res = run_bass_kernel_spmd(nc, in_maps, list(range(NCORES)))
    out = np.empty((TOK, C), dtype=np.float32)
    cz = np.empty((TOK, C), dtype=np.float32)
    for i in range(NCORES):
        out[i * TPC:(i + 1) * TPC, :] = res.results[i]["outT"].T
        cz[i * TPC:(i + 1) * TPC, :] = res.results[i]["czT"].T
    return (out.reshape(B, N, C), cz.reshape(B, N, C))



# revision 13
# speedup vs baseline: 1.5294x; 1.5294x over previous
"""Trainium2 Bass kernel for causal attention + proj + causal_features.

Problem shapes: x [2, 2048, 1024], H=16 heads, HD=64.
Strategy (8 NeuronCores):
  - Head-parallel attention: core i computes QKV + attention for heads {2i, 2i+1}
    over both batches, in transposed [channel, token] layout so the contraction
    dim sits on SBUF partitions. All matmul operands are bf16 (PSUM accumulates
    f32); rel-err budget is 2e-2 so bf16 rounding (~0.5%) is fine.
  - x is re-laid-out host-side per 512-token chunk so each chunk is ONE DMA
    with 8KB-contiguous partition rows (DMA trigger cost ~630ns each on the
    shared HWDGE, so fewer/fatter DMAs matter more than bytes).
  - Scores computed as S^T[k, q] = K @ Q^T (contraction d=64); exp on ScalarE
    (scale=1/8 folded in). Diagonal-band k-tiles are trimmed to the valid
    q-suffix (512-128t wide) and only the single [128,128] diagonal block is
    masked with a triangular multiplier; fully-masked regions are never
    computed. Upper-triangle tiles skipped entirely.
  - attn @ V via lhsT = [V | 1] (ones column interleaved) so softmax
    denominators fall out of the same matmul as row 64 of the PSUM accumulator.
    AV matmuls are ordered so the start/stop accumulation markers land on
    full-width tiles (PSUM zero-region bookkeeping).
  - AllToAll (bf16) reshards attention output from head-parallel to
    token-parallel; each core then computes proj + bias and causal_features for
    its 512-token slice. Outputs returned transposed bf16 [1024, 512] and
    re-assembled + upcast on host.
  - A few dummy matmuls at program start warm the PE HAM clock gate while the
    first DMAs stream in.
"""

import numpy as np
import ml_dtypes  # noqa: F401  (registers bfloat16 with numpy)
import concourse.bass as bass
import concourse.mybir as mybir
import concourse.tile as tile
from concourse import bacc
from concourse.bass_utils import run_bass_kernel_spmd

B, N, C, H, HD = 2, 2048, 1024, 16, 64
NCORES = 8
TOK = B * N            # 4096 global tokens
TPC = TOK // NCORES    # 512 tokens per core (output slice)
QC = 512               # q chunk width
KTILE = 128            # k tile height
NKT = N // KTILE       # 16 k tiles per batch
NQC = N // QC          # 4 q chunks per batch
NCH = TOK // QC        # 8 chunks
f32 = mybir.dt.float32
bf16 = mybir.dt.bfloat16
AF = mybir.ActivationFunctionType
ALU = mybir.AluOpType

_CACHE = {}


def _build_program(fake_a2a=False, reps=1):
    nc = bacc.Bacc("TRN2", target_bir_lowering=False, debug=False, num_devices=NCORES)

    xP_d = nc.dram_tensor("xP", [NCH, 128, 8 * QC], bf16, kind="ExternalInput")
    wqkv_d = nc.dram_tensor("wqkv", [128, 3 * 8 * 128], bf16, kind="ExternalInput")
    wp_d = nc.dram_tensor("wp", [2, 128, 4 * 1024], bf16, kind="ExternalInput")
    wc_d = nc.dram_tensor("wc", [2, 128, 4 * 1024], bf16, kind="ExternalInput")
    bias_d = nc.dram_tensor("bias", [128, 16], f32, kind="ExternalInput")
    trid_d = nc.dram_tensor("trid", [128, 192], bf16, kind="ExternalInput")
    outT_d = nc.dram_tensor("outT", [C, TPC], bf16, kind="ExternalOutput")
    czT_d = nc.dram_tensor("czT", [C, TPC], bf16, kind="ExternalOutput")

    with tile.TileContext(nc) as tc:
        with tc.tile_pool(name="sb", bufs=1) as sb, \
             tc.tile_pool(name="ps", bufs=1, space="PSUM") as ps, \
             tc.tile_pool(name="dr", bufs=1, space="DRAM") as dr:

            # ---- constants: wqkv (kt-major) in 2 halves interleaved with the
            #      first x slab halves so QKV kt 0-3 can start early ----
            wqkv_sb = sb.tile([128, 3 * 8 * 128], bf16)
            nc.sync.dma_start(wqkv_sb[:, 0:1536], wqkv_d[:, 0:1536])
            nc.sync.dma_start(wqkv_sb[:, 1536:3072], wqkv_d[:, 1536:3072])
            trid_sb = sb.tile([128, 192], bf16)
            nc.gpsimd.dma_start(trid_sb[:], trid_d[:])
            bias_sb = sb.tile([128, 16], f32)       # cols 0:8 = bp tiles, 8:16 = bc
            nc.gpsimd.dma_start(bias_sb[:], bias_d[:])
            tri = trid_sb[:, 0:128]                 # triangular diag-block mask
            ident = trid_sb[:, 128:192]             # 2x stacked 64-identity

            # ---- PE warmup: dummy matmuls on a zeroed tile release the HAM
            #      clock throttle while wqkv/x stream in ----
            warm = sb.tile([128, 640], bf16)
            nc.vector.memset(warm[:], 0.0)
            for w in range(10):
                wps = ps.tile([128, 512], f32, tag="mm", bufs=3, name=f"warm_{w}")
                nc.tensor.matmul(wps[:], warm[:, 0:128], warm[:, 128:640],
                                 start=True, stop=True)

            for _rep in range(reps):
                qT_sb = sb.tile([128, TOK], bf16, tag="qT", bufs=1, name="qT_sb")
                kT_sb = sb.tile([128, TOK], bf16, tag="kT", bufs=1, name="kT_sb")
                v_aug = sb.tile([128, B * 2 * NKT * 65], bf16, tag="v_aug", bufs=1,
                                name="v_aug")
                outT_sb = sb.tile([128, TOK], bf16, tag="ot", bufs=1, name="outT_sb")
                # ones columns of v_aug (softmax denominator trick), written once
                nc.vector.tensor_scalar(v_aug[:, 64::65], warm[:, 0:B * 2 * NKT],
                                        0.0, 1.0, ALU.mult, ALU.add)

                wpms, wcms = {}, {}
                a2a_in = dr.tile([NCORES, 128, TPC], bf16, name="a2a_in")
                a2a_out = dr.tile([NCORES, 128, TPC], bf16, name="a2a_out")

                # ---- fused pipeline over token chunks ----
                for n in range(NCH):
                    b, j = n // NQC, n % NQC
                    # one fat DMA per chunk (8KB contiguous per partition)
                    xs = sb.tile([128, 8 * QC], bf16, tag="xs", bufs=2,
                                 name=f"xs_{n}")
                    dma_eng = nc.sync if n % 2 == 0 else nc.scalar
                    if n == 0:
                        # split the first slab so QKV kt 0-3 can start sooner
                        nc.scalar.dma_start(xs[:, 0:4 * QC], xP_d[0][:, 0:4 * QC])
                        nc.scalar.dma_start(xs[:, 4 * QC:8 * QC],
                                            xP_d[0][:, 4 * QC:8 * QC])
                    else:
                        dma_eng.dma_start(xs[:], xP_d[n])
                    # QKV for chunk n (wqkv is kt-major: slice (kt*3+m))
                    accs = [ps.tile([128, QC], f32, tag="mm", bufs=3,
                                    name=f"qkv_{n}_{m}") for m in range(3)]
                    for kt in range(8):
                        for m in range(3):
                            nc.tensor.matmul(
                                accs[m][:],
                                wqkv_sb[:, (kt * 3 + m) * 128:(kt * 3 + m + 1) * 128],
                                xs[:, kt * QC:(kt + 1) * QC],
                                start=(kt == 0), stop=(kt == 7))
                    nc.vector.tensor_copy(qT_sb[:, n * QC:(n + 1) * QC], accs[0][:])
                    nc.vector.tensor_copy(kT_sb[:, n * QC:(n + 1) * QC], accs[1][:])
                    vt = sb.tile([128, QC], bf16, tag="vt", bufs=2, name=f"vt_{n}")
                    nc.vector.tensor_copy(vt[:], accs[2][:])
                    # V transposes for this chunk into v_aug slots
                    for h in range(2):
                        tp4 = ps.tile([128, 256], bf16, tag="mm", bufs=3,
                                      name=f"vt_{n}_{h}")
                        for t in range(4):
                            nc.tensor.transpose(
                                tp4[:, t * 64:(t + 1) * 64],
                                vt[h * 64:(h + 1) * 64, t * 128:(t + 1) * 128],
                                ident[h * 64:(h + 1) * 64, :])
                        for t in range(4):
                            kt = 4 * j + t
                            slot = ((b * 2 + h) * NKT + kt) * 65
                            nc.vector.tensor_copy(v_aug[:, slot:slot + 64],
                                                  tp4[:, t * 64:(t + 1) * 64])

                    # ---- attention for (b, j) ----
                    # step list: full-tile pairs, then diagonal tiles ordered so
                    # the first and last AV matmuls are full-width (PSUM
                    # zero-region start/stop must cover the whole av tile).
                    qc0 = b * N + j * QC
                    if j == 0:
                        diag_order = [0, 1, 2, 3]
                    else:
                        diag_order = [1, 2, 3, 0]
                    steps = [("pair", g) for g in range(2 * j)] + \
                            [("diag", t) for t in diag_order]
                    nsteps = len(steps)
                    for h in range(2):
                        av = ps.tile([65, QC], f32, tag="av", bufs=1,
                                     name=f"av_{n}_{h}")
                        pend = {}
                        TRAIL = 2
                        for si in range(nsteps + TRAIL):
                            if si < nsteps:
                                kind, arg = steps[si]
                                if kind == "pair":
                                    kt0 = 2 * arg
                                    sc2 = ps.tile([128, 2 * QC], f32, tag="sc",
                                                  bufs=2, name=f"sc_{n}_{h}_{si}")
                                    for t in range(2):
                                        kk = b * N + (kt0 + t) * KTILE
                                        nc.tensor.matmul(
                                            sc2[:, t * QC:(t + 1) * QC],
                                            kT_sb[h * 64:(h + 1) * 64, kk:kk + 128],
                                            qT_sb[h * 64:(h + 1) * 64, qc0:qc0 + QC],
                                            start=True, stop=True)
                                    ex = sb.tile([128, 2 * QC], bf16, tag="ex",
                                                 bufs=6, name=f"ex_{n}_{h}_{si}")
                                    nc.scalar.activation(ex[:], sc2[:], AF.Exp,
                                                         scale=HD ** -0.5)
                                else:
                                    t = arg
                                    kt = 4 * j + t
                                    w = QC - t * 128
                                    kk = b * N + kt * KTILE
                                    scd = ps.tile([128, w], f32, tag="sc", bufs=2,
                                                  name=f"sc_{n}_{h}_{si}")
                                    nc.tensor.matmul(
                                        scd[:],
                                        kT_sb[h * 64:(h + 1) * 64, kk:kk + 128],
                                        qT_sb[h * 64:(h + 1) * 64,
                                              qc0 + t * 128:qc0 + QC],
                                        start=True, stop=True)
                                    if j == 0:
                                        # full-width ex with zeroed prefix so the
                                        # AV matmul can stay full-width
                                        ex = sb.tile([128, QC], bf16, tag="ex",
                                                     bufs=6, name=f"ex_{n}_{h}_{si}")
                                        if t > 0:
                                            nc.vector.memset(ex[:, 0:t * 128], 0.0)
                                        nc.scalar.activation(ex[:, t * 128:QC],
                                                             scd[:], AF.Exp,
                                                             scale=HD ** -0.5)
                                        nc.vector.tensor_tensor(
                                            ex[:, t * 128:(t + 1) * 128],
                                            ex[:, t * 128:(t + 1) * 128],
                                            tri, ALU.mult)
                                    else:
                                        ex = sb.tile([128, w], bf16, tag="ex",
                                                     bufs=6, name=f"ex_{n}_{h}_{si}")
                                        nc.scalar.activation(ex[:], scd[:], AF.Exp,
                                                             scale=HD ** -0.5)
                                        nc.vector.tensor_tensor(
                                            ex[:, 0:128], ex[:, 0:128], tri,
                                            ALU.mult)
                                pend[si] = ex
                            # AV trails so PE doesn't in-order block on exp
                            ai = si - TRAIL
                            if 0 <= ai < nsteps:
                                kind, arg = steps[ai]
                                exa = pend.pop(ai)
                                first = ai == 0
                                last = ai == nsteps - 1
                                if kind == "pair":
                                    kt0 = 2 * arg
                                    for t in range(2):
                                        kt = kt0 + t
                                        slot = ((b * 2 + h) * NKT + kt) * 65
                                        nc.tensor.matmul(
                                            av[:], v_aug[:, slot:slot + 65],
                                            exa[:, t * QC:(t + 1) * QC],
                                            start=(first and t == 0),
                                            stop=(last and t == 1))
                                else:
                                    t = arg
                                    kt = 4 * j + t
                                    slot = ((b * 2 + h) * NKT + kt) * 65
                                    if j == 0 or t == 0:
                                        nc.tensor.matmul(
                                            av[:], v_aug[:, slot:slot + 65],
                                            exa[:, 0:QC] if j == 0 else exa[:],
                                            start=first, stop=last)
                                    else:
                                        w = QC - t * 128
                                        nc.tensor.matmul(
                                            av[:, t * 128:QC],
                                            v_aug[:, slot:slot + 65],
                                            exa[:, 0:w],
                                            start=False, stop=False)
                        rc = sb.tile([1, QC], f32, tag="rc", bufs=2,
                                     name=f"rc_{n}_{h}")
                        with nc.allow_low_precision(reason="softmax denom recip"):
                            nc.vector.reciprocal(rc[:], av[64:65, :])
                        rb = sb.tile([64, QC], f32, tag="rb", bufs=2,
                                     name=f"rb_{n}_{h}")
                        nc.gpsimd.partition_broadcast(rb[:], rc[:])
                        nc.vector.tensor_tensor(
                            outT_sb[h * 64:(h + 1) * 64, qc0:qc0 + QC],
                            av[0:64, :], rb[:], ALU.mult)

                    # stage this chunk's A2A slice (chunk n == dest core n)
                    nc.gpsimd.dma_start(a2a_in[n], outT_sb[:, n * TPC:(n + 1) * TPC])
                    if n == 5:
                        # prefetch proj/causal weights under the attention tail
                        for half in range(2):
                            wpm = sb.tile([128, 4096], bf16, tag="wp", bufs=2,
                                          name=f"wpm_{half}")
                            nc.gpsimd.dma_start(wpm[:], wp_d[half])
                            wpms[half] = wpm
                            wcm = sb.tile([128, 4096], bf16, tag="wc", bufs=2,
                                          name=f"wcm_{half}")
                            nc.gpsimd.dma_start(wcm[:], wc_d[half])
                            wcms[half] = wcm

                # ---- AllToAll: head-parallel -> token-parallel ----
                if fake_a2a:
                    nc.sync.dma_start(a2a_out[:], a2a_in[:])
                else:
                    nc.gpsimd.collective_compute(
                        "AllToAll", ALU.bypass,
                        replica_groups=[list(range(NCORES))],
                        ins=[a2a_in.opt()], outs=[a2a_out.opt()])
                otf = sb.tile([128, NCORES * TPC], bf16, tag="otf", bufs=1,
                              name="otf")
                nc.sync.dma_start(otf[:],
                                  a2a_out[:].rearrange("d p c -> p d c"))

                # ---- proj + bias (projT [c_out, tok]) ----
                pj_sb = sb.tile([128, 8 * TPC], bf16, tag="pj", bufs=1, name="pj_sb")
                for m in range(8):
                    wpm = wpms[m // 4]
                    c0 = (m % 4) * 1024
                    acc = ps.tile([128, TPC], f32, tag="sc", bufs=2,
                                  name=f"pj_ps_{m}")
                    for kt in range(8):
                        nc.tensor.matmul(acc[:],
                                         wpm[:, c0 + kt * 128:c0 + (kt + 1) * 128],
                                         otf[:, kt * TPC:(kt + 1) * TPC],
                                         start=(kt == 0), stop=(kt == 7))
                    nc.vector.tensor_scalar_add(pj_sb[:, m * TPC:(m + 1) * TPC],
                                                acc[:], bias_sb[:, m:m + 1])
                    if m % 4 == 3:
                        half = m // 4
                        nc.sync.dma_start(
                            outT_d[half * 512:(half + 1) * 512, :]
                            .rearrange("(m p) c -> p m c", m=4),
                            pj_sb[:, half * 4 * TPC:(half + 1) * 4 * TPC])

                # ---- causal_features + bias ----
                cz_sb = sb.tile([128, 8 * TPC], bf16, tag="cz", bufs=1, name="cz_sb")
                for m in range(8):
                    wcm = wcms[m // 4]
                    c0 = (m % 4) * 1024
                    acc = ps.tile([128, TPC], f32, tag="sc", bufs=2,
                                  name=f"cz_ps_{m}")
                    for kt in range(8):
                        nc.tensor.matmul(acc[:],
                                         wcm[:, c0 + kt * 128:c0 + (kt + 1) * 128],
                                         pj_sb[:, kt * TPC:(kt + 1) * TPC],
                                         start=(kt == 0), stop=(kt == 7))
                    nc.vector.tensor_scalar_add(cz_sb[:, m * TPC:(m + 1) * TPC],
                                                acc[:], bias_sb[:, 8 + m:9 + m])
                    eng = nc.scalar if m % 2 == 0 else nc.sync
                    eng.dma_start(czT_d[m * 128:(m + 1) * 128, :],
                                  cz_sb[:, m * TPC:(m + 1) * TPC])

    nc.finalize()
    return nc


def _pack_w(w):
    # [kt*128+p, m*128+c] -> [half, p, (m%4)*1024 + kt*128 + c]
    w = np.asarray(w, dtype=np.float32).reshape(8, 128, 8, 128)   # [kt, p, m, c]
    w = w.transpose(2, 1, 0, 3).reshape(8, 128, 1024)             # [m, p, kt*128+c]
    w = w.reshape(2, 4, 128, 1024).transpose(0, 2, 1, 3)          # [half, p, m', ktc]
    return np.ascontiguousarray(w.reshape(2, 128, 4096)).astype(ml_dtypes.bfloat16)


def _host_inputs(x, mask, W_qkv, W_proj, b_proj, W_causal, b_causal):
    x = np.asarray(x, dtype=np.float32)
    # chunk-contiguous layout: xP[n, p, kt*512 + t] = x_tok[n*512 + t, kt*128 + p]
    xt = x.reshape(TOK, C).reshape(NCH, QC, 8, 128)
    xP = np.ascontiguousarray(xt.transpose(0, 3, 2, 1).reshape(NCH, 128, 8 * QC))
    xP = xP.astype(ml_dtypes.bfloat16)
    m2 = np.asarray(mask).reshape(N, N)
    # triangular diag-block mask in S^T [k, q] layout
    tri = m2[0:128, 0:128].T.astype(np.float32)
    ident = np.concatenate([np.eye(64, dtype=np.float32)] * 2, axis=0)
    trid = np.ascontiguousarray(
        np.concatenate([tri, ident], axis=1)).astype(ml_dtypes.bfloat16)
    W_qkv = np.asarray(W_qkv, dtype=np.float32)
    shared = {
        "xP": xP,
        "wp": _pack_w(W_proj),
        "wc": _pack_w(W_causal),
        "bias": np.ascontiguousarray(np.stack(
            [np.asarray(b_proj, np.float32).reshape(8, 128),
             np.asarray(b_causal, np.float32).reshape(8, 128)])
            .transpose(2, 0, 1).reshape(128, 16)),
        "trid": trid,
    }
    in_maps = []
    for i in range(NCORES):
        im = dict(shared)
        sl = np.stack([W_qkv[:, m * C + i * 128:m * C + (i + 1) * 128]
                       for m in range(3)])
        # [m, kt*128+p, c] -> [p, kt, m, c]  (kt-major so half-DMAs cover kt 0-3)
        sl = sl.reshape(3, 8, 128, 128).transpose(2, 1, 0, 3).reshape(128, 3 * 8 * 128)
        im["wqkv"] = np.ascontiguousarray(sl).astype(ml_dtypes.bfloat16)
        in_maps.append(im)
    return in_maps


def kernel(x, mask, W_qkv, W_proj, b_proj, W_causal, b_causal):
    if "nc" not in _CACHE:
        _CACHE["nc"] = _build_program()
    nc = _CACHE["nc"]
    in_maps = _host_inputs(x, mask, W_qkv, W_proj, b_proj, W_causal, b_causal)
    res = run_bass_kernel_spmd(nc, in_maps, list(range(NCORES)))
    out = np.empty((TOK, C), dtype=np.float32)
    cz = np.empty((TOK, C), dtype=np.float32)
    for i in range(NCORES):
        out[i * TPC:(i + 1) * TPC, :] = res.results[i]["outT"].astype(np.float32).T
        cz[i * TPC:(i + 1) * TPC, :] = res.results[i]["czT"].astype(np.float32).T
    return (out.reshape(B, N, C), cz.reshape(B, N, C))
